# revision 1
# baseline (speedup 1.0000x reference)
"""Child-Sum TreeLSTM over complete binary trees — Trainium2 Bass kernel.

Sharding: data-parallel over the batch-of-trees axis B=32 across 8 NeuronCores
(4 trees/core); the 8 gate weight matrices are replicated.

Per-core dataflow (activations kept feature-transposed in SBUF as
[feat-chunk(128,128,44), 3, cols] tiles; weights natural = lhsT):
  - level-by-level bottom-up; per <=512-column block:
      embs^T loaded by XBAR DMA-transpose directly from a host-prepared
      bf16 padded copy of embs ([..., 384] with feature 300 = 1.0 ones row)
      for levels >= 7; PE-transpose path for the small deep levels
      gate pre-acts accumulate in PSUM over 6 K-chunks: x-side bf16 +
      h-side float32r; the combined bias (bx+bh) rides as a 45th weight row
      against the baked-in ones row
      sigma/tanh evacuate PSUM->SBUF in one ACT instruction per gate
      per-child forget gates use a step-0 duplicated rhs (each parent column
      streamed twice) so fx lands directly at child granularity
      c_new = i*u + f1*c1 + f2*c2 and h = o*tanh(c) on DVE
      h^T -> PE-transpose -> natural -> DMA to output
  - levels 10/9/8 spill h^T/c^T through internal DRAM (SBUF pressure);
    levels <= 7 keep h^T/c^T resident in SBUF
  - matmuls with moving dim < 256 (deep levels) switch the h-side to bf16
    (float32r drops to 4 cycles/row below 256).
"""

import numpy as np
import ml_dtypes

import concourse.bass as bass
import concourse.mybir as mybir
import concourse.tile as tile
from concourse import bacc
from concourse.masks import make_identity
from concourse.bass_utils import run_bass_kernel_spmd

F32 = mybir.dt.float32
F32R = mybir.dt.float32r
BF16 = mybir.dt.bfloat16
AF = mybir.ActivationFunctionType

B, D, DIM = 32, 11, 300
N = 2**D - 1          # 2047
CORES = 8
BL = B // CORES       # trees per core
KS = [128, 128, 44]   # feature chunks of 300
KO = [0, 128, 256]
NBMAX = 512
SPILL_LV = (10, 9, 8)
SPOFF = {10: 0, 9: BL * 1024, 8: BL * 1024 + BL * 512}
SPTOT = BL * 1024 + BL * 512 + BL * 256
PROJ = {"i": 0, "f": 1, "o": 2, "u": 3}

_NC_CACHE = []


def _cols(l):
    return BL * (1 << l)


def _build():
    nc = bacc.Bacc("TRN2", target_bir_lowering=False, debug=False,
                   num_devices=CORES)
    embs = nc.dram_tensor("embs", [BL, N, DIM], F32, kind="ExternalInput")
    WX = nc.dram_tensor("wx", [128, 4, 3, DIM], F32R, kind="ExternalInput")
    WH = nc.dram_tensor("wh", [128, 4, 3, DIM], F32R, kind="ExternalInput")
    hout = nc.dram_tensor("hout", [BL, N, DIM], F32, kind="ExternalOutput")
    sph = nc.dram_tensor("sph", [128, 3, SPTOT], F32R)
    spc = nc.dram_tensor("spc", [128, 3, SPTOT], F32)

    with tile.TileContext(nc) as tc:
        import contextlib
        with contextlib.ExitStack() as ctx:
            sb = ctx.enter_context(tc.tile_pool(name="sb", bufs=1))
            exp = ctx.enter_context(tc.tile_pool(name="exp", bufs=2))
            xtp = ctx.enter_context(tc.tile_pool(name="xtp", bufs=2))
            hsp = ctx.enter_context(tc.tile_pool(name="hsp", bufs=2))
            gp = ctx.enter_context(tc.tile_pool(name="gp", bufs=5))
            fcp = ctx.enter_context(tc.tile_pool(name="fcp", bufs=2))
            onp = ctx.enter_context(tc.tile_pool(name="onp", bufs=2))
            hcb = ctx.enter_context(tc.tile_pool(name="hcb", bufs=2))
            rbp = ctx.enter_context(tc.tile_pool(name="rbp", bufs=2))
            stp = ctx.enter_context(tc.tile_pool(name="stp", bufs=1))
            psum = ctx.enter_context(
                tc.tile_pool(name="psum", bufs=2, space="PSUM"))

            ident = sb.tile([128, 128], F32)
            make_identity(nc, ident[:, :])

            wx_t = sb.tile([128, 4, 3, DIM], F32R, name="wx_t")
            wh_t = sb.tile([128, 4, 3, DIM], F32R, name="wh_t")
            nc.sync.dma_start(out=wx_t[:, :, :, :], in_=WX[:, :, :, :])
            nc.sync.dma_start(out=wh_t[:, :, :, :], in_=WH[:, :, :, :])
            wx = {nm: wx_t[:, p] for nm, p in PROJ.items()}
            wh = {nm: wh_t[:, p] for nm, p in PROJ.items()}

            def nat_ap(dram, l, r0, rs, w):
                base = (1 << l) - 1
                if l >= 7:
                    t, j0 = r0 >> l, r0 & ((1 << l) - 1)
                    return dram[t, base + j0: base + j0 + rs, 0:w]
                t0, tcnt = r0 >> l, rs >> l
                return dram[t0:t0 + tcnt, base:base + (1 << l), 0:w]

            def load_ex(l, c0, nb):
                """embs^T (+ones row) for parent cols [c0, c0+nb), bf16."""
                ex = exp.tile([128, 3, NBMAX], F32R, tag="ex")
                pT = psum.tile([128, 3, NBMAX], F32, tag="big")
                for r0 in range(0, nb, 128):
                    rs = min(128, nb - r0)
                    xt = xtp.tile([128, 304], F32, tag="xt")
                    nc.gpsimd.memset(xt[:, 300:304], 1.0)
                    nc.sync.dma_start(out=xt[0:rs, 0:300],
                                      in_=nat_ap(embs, l, c0 + r0, rs, DIM))
                    for f in range(3):
                        ke = KS[f] + (1 if f == 2 else 0)
                        nc.tensor.transpose(
                            out=pT[0:ke, f, r0:r0 + rs],
                            in_=xt[0:rs, KO[f]:KO[f] + ke],
                            identity=ident[0:rs, 0:rs])
                nc.scalar.copy(ex[0:128, 0, 0:nb], pT[0:128, 0, 0:nb])
                nc.scalar.copy(ex[0:128, 1, 0:nb], pT[0:128, 1, 0:nb])
                nc.scalar.copy(ex[0:45, 2, 0:nb], pT[0:45, 2, 0:nb])
                return ex

            def store_nat(l, c0, nb, hsrc, s0):
                for r0 in range(0, nb, 128):
                    rs = min(128, nb - r0)
                    pO = psum.tile([128, 304], F32, tag="oT")
                    for f in range(3):
                        nc.tensor.transpose(
                            out=pO[0:rs, KO[f]:KO[f] + KS[f]],
                            in_=hsrc[0:KS[f], f,
                                     s0 + r0:s0 + r0 + rs].bitcast(F32),
                            identity=ident[0:KS[f], 0:KS[f]])
                    on = onp.tile([128, 300], F32, tag="on")
                    nc.scalar.copy(on[0:rs, :], pO[0:rs, 0:300])
                    nc.gpsimd.dma_start(out=nat_ap(hout, l, c0 + r0, rs,
                                                   DIM),
                                        in_=on[0:rs, :])

            st_h = {l: stp.tile([128, 3, _cols(l)], F32R, tag=f"sh{l}",
                                name=f"sh{l}") for l in range(0, 8)}
            st_c = {l: stp.tile([128, 3, _cols(l)], F32, tag=f"sc{l}",
                                name=f"sc{l}") for l in range(0, 8)}

            # ---------------- leaves (level 10) ----------------
            def leaf_block(c0):
                l, nb = 10, NBMAX
                ex = load_ex(l, c0, nb)
                sg = {}
                for nm, fn in (("i", AF.Sigmoid), ("o", AF.Sigmoid),
                               ("u", AF.Tanh)):
                    pG = psum.tile([128, 3, NBMAX], F32, tag="big",
                                   name=f"lpg_{c0}_{nm}")
                    for m in range(3):
                        ms, mo = KS[m], KO[m]
                        for k in range(3):
                            kx = KS[k] + (1 if k == 2 else 0)
                            nc.tensor.matmul(
                                pG[0:ms, m, 0:nb],
                                wx[nm][0:kx, k, mo:mo + ms],
                                ex[0:kx, k, 0:nb],
                                start=(k == 0), stop=(k == 2))
                    g = gp.tile([128, 3, NBMAX], F32, tag="g",
                                name=f"lg_{c0}_{nm}")
                    nc.scalar.activation(g[:, :, 0:nb], pG[:, :, 0:nb], fn)
                    sg[nm] = g
                cb = hcb.tile([128, 3, NBMAX], F32, tag="lc", bufs=3,
                              name=f"lc_{c0}")
                hb = hcb.tile([128, 3, NBMAX], F32R, tag="lh", bufs=3,
                              name=f"lh_{c0}")
                nc.vector.tensor_mul(cb[:, :, 0:nb], sg["i"][:, :, 0:nb],
                                     sg["u"][:, :, 0:nb])
                th = gp.tile([128, 3, NBMAX], F32, tag="g",
                             name=f"lth_{c0}")
                nc.scalar.activation(th[:, :, 0:nb], cb[:, :, 0:nb], AF.Tanh)
                nc.vector.tensor_mul(hb[:, :, 0:nb], sg["o"][:, :, 0:nb],
                                     th[:, :, 0:nb])
                store_nat(l, c0, nb, hb, 0)
                return hb, cb

            # ---------------- internal levels 9..0 ----------------
            for l in range(9, -1, -1):
                cols = _cols(l)
                spill = l in SPILL_LV
                child_spill = (l + 1) in SPILL_LV
                for c0 in range(0, cols, NBMAX):
                    nb = min(NBMAX, cols - c0)
                    fs = min(2 * nb, NBMAX)
                    nsub = (2 * nb) // fs
                    if l == 9:
                        leaf_hc = [leaf_block(2 * c0 + s * fs)
                                   for s in range(nsub)]
                    ex = load_ex(l, c0, nb)

                    hn, cn = [], []
                    for s in range(nsub):
                        ch0 = 2 * c0 + s * fs
                        if l == 9:
                            hn.append((leaf_hc[s][0], 0))
                            cn.append((leaf_hc[s][1], 0))
                        elif child_spill:
                            t_h = rbp.tile([128, 3, NBMAX], F32R, tag="rh")
                            t_c = rbp.tile([128, 3, NBMAX], F32, tag="rc")
                            off = SPOFF[l + 1] + ch0
                            nc.sync.dma_start(out=t_h[:, :, 0:fs],
                                              in_=sph[:, :, off:off + fs])
                            nc.sync.dma_start(out=t_c[:, :, 0:fs],
                                              in_=spc[:, :, off:off + fs])
                            hn.append((t_h, 0))
                            cn.append((t_c, 0))
                        else:
                            hn.append((st_h[l + 1], ch0))
                            cn.append((st_c[l + 1], ch0))

                    hs = hsp.tile([128, 3, NBMAX], F32R, tag="hs",
                                  name=f"hs_{l}_{c0}")
                    for s in range(nsub):
                        t_h, o_h = hn[s]
                        pair = t_h[:, :, o_h:o_h + fs].rearrange(
                            "p c (n two) -> p c n two", two=2)
                        nc.vector.tensor_add(
                            hs[:, :, s * fs // 2:(s + 1) * fs // 2],
                            pair[:, :, :, 0], pair[:, :, :, 1])

                    sg = {}
                    for nm, fn in (("i", AF.Sigmoid), ("o", AF.Sigmoid),
                                   ("u", AF.Tanh)):
                        pG = psum.tile([128, 3, NBMAX], F32, tag="big")
                        for m in range(3):
                            ms, mo = KS[m], KO[m]
                            for k in range(3):
                                kx = KS[k] + (1 if k == 2 else 0)
                                nc.tensor.matmul(
                                    pG[0:ms, m, 0:nb],
                                    wx[nm][0:kx, k, mo:mo + ms],
                                    ex[0:kx, k, 0:nb],
                                    start=(k == 0), stop=False)
                            for k in range(3):
                                nc.tensor.matmul(
                                    pG[0:ms, m, 0:nb],
                                    wh[nm][0:KS[k], k, mo:mo + ms],
                                    hs[0:KS[k], k, 0:nb],
                                    start=False, stop=(k == 2))
                        g = gp.tile([128, 3, NBMAX], F32, tag="g")
                        nc.scalar.activation(g[:, :, 0:nb], pG[:, :, 0:nb], fn)
                        sg[nm] = g

                    if spill:
                        cdst = hcb.tile([128, 3, NBMAX], F32, tag="cb")
                        hdst = hcb.tile([128, 3, NBMAX], F32R, tag="hb")
                        d0 = 0
                    else:
                        cdst, hdst, d0 = st_c[l], st_h[l], c0

                    cc = cdst[:, :, d0:d0 + nb]
                    nc.vector.tensor_mul(cc, sg["i"][:, :, 0:nb],
                                         sg["u"][:, :, 0:nb])

                    for s in range(nsub):
                        pF = psum.tile([128, 3, NBMAX], F32, tag="big")
                        p0 = s * fs // 2
                        w_h = wh["f"]
                        t_h, o_h = hn[s]
                        for m in range(3):
                            ms, mo = KS[m], KO[m]
                            for k in range(3):
                                kx = KS[k] + (1 if k == 2 else 0)
                                dup = ex[0:kx, k, p0:p0 + fs // 2] \
                                    .unsqueeze(2).broadcast_to([kx, fs // 2, 2])
                                nc.tensor.matmul(
                                    pF[0:ms, m, 0:fs],
                                    wx["f"][0:kx, k, mo:mo + ms], dup,
                                    start=(k == 0), stop=False)
                            for k in range(3):
                                nc.tensor.matmul(
                                    pF[0:ms, m, 0:fs],
                                    w_h[0:KS[k], k, mo:mo + ms],
                                    t_h[0:KS[k], k, o_h:o_h + fs],
                                    start=False, stop=(k == 2))
                        fg = gp.tile([128, 3, NBMAX], F32, tag="g")
                        nc.scalar.activation(fg[:, :, 0:fs], pF[:, :, 0:fs],
                                             AF.Sigmoid)
                        t_c, o_c = cn[s]
                        fc = fcp.tile([128, 3, NBMAX], F32, tag="fc")
                        nc.vector.tensor_mul(fc[:, :, 0:fs],
                                             fg[:, :, 0:fs],
                                             t_c[:, :, o_c:o_c + fs])
                        pair = fc[:, :, 0:fs].rearrange(
                            "p c (n two) -> p c n two", two=2)
                        ccs = cdst[:, :, d0 + p0:d0 + p0 + fs // 2]
                        nc.vector.tensor_add(ccs, ccs, pair[:, :, :, 0])
                        nc.vector.tensor_add(ccs, ccs, pair[:, :, :, 1])

                    th = gp.tile([128, 3, NBMAX], F32, tag="g")
                    nc.scalar.activation(th[:, :, 0:nb], cc, AF.Tanh)
                    nc.vector.tensor_mul(hdst[:, :, d0:d0 + nb],
                                         sg["o"][:, :, 0:nb], th[:, :, 0:nb])

                    if spill:
                        off = SPOFF[l] + c0
                        nc.gpsimd.dma_start(out=sph[:, :, off:off + nb],
                                            in_=hdst[:, :, 0:nb])
                        nc.gpsimd.dma_start(out=spc[:, :, off:off + nb],
                                            in_=cdst[:, :, 0:nb])
                    store_nat(l, c0, nb, hdst, d0)
    nc.compile()
    return nc


def kernel(embs, Wix, bix, Wih, bih, Wfx, bfx, Wfh, bfh,
           Wox, box, Woh, boh, Wux, bux, Wuh, buh):
    embs = np.ascontiguousarray(np.asarray(embs, dtype=np.float32))
    if not _NC_CACHE:
        _NC_CACHE.append(_build())
    nc = _NC_CACHE[0]

    def chunked(stack, bias_rows):
        out = np.zeros((128, 4, 3, DIM), np.float32)
        for p in range(4):
            out[0:128, p, 0] = stack[p][0:128]
            out[0:128, p, 1] = stack[p][128:256]
            out[0:44, p, 2] = stack[p][256:300]
            if bias_rows is not None:
                out[44, p, 2] = bias_rows[p]
        return out

    xw = [np.asarray(w, np.float32) for w in (Wix, Wfx, Wox, Wux)]
    xb = [np.asarray(bix) + np.asarray(bih), np.asarray(bfx) + np.asarray(bfh),
          np.asarray(box) + np.asarray(boh), np.asarray(bux) + np.asarray(buh)]
    hw_ = [np.asarray(w, np.float32) for w in (Wih, Wfh, Woh, Wuh)]
    wxp = chunked(xw, xb)
    whp = chunked(hw_, None)

    in_maps = [{"embs": embs[c * BL:(c + 1) * BL],
                "wx": wxp, "wh": whp}
               for c in range(CORES)]
    res = run_bass_kernel_spmd(nc, in_maps, list(range(CORES)))
    return np.concatenate([res.results[c]["hout"] for c in range(CORES)],
                          axis=0)



# revision 9
# speedup vs baseline: 1.7436x; 1.7436x over previous
"""Child-Sum TreeLSTM over complete binary trees — Trainium2 Bass kernel.

Sharding: data-parallel over the batch-of-trees axis B=32 across 8 NeuronCores
(4 trees/core); the 8 gate weight matrices are replicated.

Layout: per level, columns are permuted by per-level bit-reversal (col =
bitrev_l(j)*4 + tree).  Under this order the children of parent col p at
level l sit at cols p (left) and cols_l + p (right) of level l+1 — children
are split into two contiguous halves, so child-sum, per-child forget gates
and f*c reductions are all plain packed slices (no strided pairs, no
broadcast rhs).  The host prepares feature-major bf16 copies of embs in this
order and un-permutes h on the way out, so the kernel does no transposes.

Per-core dataflow, all matmuls bf16 (1 cycle/col):
  rhs tiles per level: X (x rows 0..255), and three M tiles (iou / f-left /
  f-right) holding chunk0 = [h rows 256..299 | x rows 256..299 | ones] and
  chunks 1,2 = h rows 0..127 / 128..255.  The ones lane carries the combined
  bias (bx+bh) as a weight row, so gate preacts finish in 5 matmul passes
  (3 for leaves) per (gate, m-chunk), PSUM-accumulated.
  PSUM evacuation with sigmoid/tanh on ACT; all elementwise work uses
  scalar_tensor_tensor (InstTensorScalarPtr, 4x DVE perf mode on packed
  bf16 SBUF).  h is written straight into the parent level's f-rhs tiles;
  h_sum is one packed add.  Everything stays SBUF-resident (no spills).
"""

import numpy as np
import ml_dtypes

import concourse.bass as bass
import concourse.mybir as mybir
import concourse.tile as tile
from concourse import bacc
from concourse.bass_utils import run_bass_kernel_spmd

F32 = mybir.dt.float32
BF16 = mybir.dt.bfloat16
AF = mybir.ActivationFunctionType
ALU = mybir.AluOpType

B, D, DIM = 32, 11, 300
N = 2**D - 1            # 2047
CORES = 8
BL = B // CORES         # 4 trees per core
NCOLS = BL * N          # 8188
NB = 512
LF = D - 1              # leaf level = 10
MCH = ((0, 128), (128, 128), (256, 44))   # m-chunks of 300

_NC_CACHE = []
_PREP_CACHE = {}


def _cols(l):
    return BL * (1 << l)


def _off(l):
    return BL * ((1 << l) - 1)


def _build():
    nc = bacc.Bacc("TRN2", target_bir_lowering=False, debug=False,
                   num_devices=CORES)
    X = nc.dram_tensor("x", [128, 2, NCOLS], BF16, kind="ExternalInput")
    XT = nc.dram_tensor("xt", [65, NCOLS], BF16, kind="ExternalInput")
    W = nc.dram_tensor("w", [128, 4, 5, DIM], BF16, kind="ExternalInput")
    HT = nc.dram_tensor("ht", [128, 3, NCOLS], BF16, kind="ExternalOutput")

    GI, GO, GU, GF = 0, 1, 2, 3

    with tile.TileContext(nc) as tc:
        import contextlib
        with contextlib.ExitStack() as ctx:
            sb = ctx.enter_context(tc.tile_pool(name="sb", bufs=1))
            gp = ctx.enter_context(tc.tile_pool(name="gp", bufs=2))
            psum = ctx.enter_context(
                tc.tile_pool(name="psum", bufs=2, space="PSUM"))

            wt = sb.tile([128, 4, 5, DIM], BF16, name="wt")
            for g in range(4):
                nc.sync.dma_start(out=wt[:, g], in_=W[:, g])

            # per-level tiles
            mx = {l: sb.tile([128, 2, _cols(l)], BF16, name=f"mx{l}",
                             tag=f"mx{l}") for l in range(0, LF)}
            mio = {l: sb.tile([128, 3, _cols(l)], BF16, name=f"mio{l}",
                              tag=f"mio{l}") for l in range(0, LF)}
            mfl = {l: sb.tile([128, 3, _cols(l)], BF16, name=f"mfl{l}",
                              tag=f"mfl{l}") for l in range(0, LF)}
            mfr = {l: sb.tile([128, 3, _cols(l)], BF16, name=f"mfr{l}",
                              tag=f"mfr{l}") for l in range(0, LF)}
            lx = sb.tile([128, 2, _cols(LF)], BF16, name="lx")
            ct = {l: sb.tile([128, 3, _cols(l)], BF16, name=f"ct{l}",
                             tag=f"ct{l}") for l in range(0, LF + 1)}
            hroot = sb.tile([128, 3, BL], BF16, name="hroot")

            def stt(out, in0, in1, op1):
                # tensor_tensor gets the 2x_1p DVE perf mode on packed bf16
                # (scalar_tensor_tensor supports no perf modes)
                if op1 == ALU.add:
                    nc.vector.tensor_add(out, in0, in1)
                else:
                    nc.vector.tensor_mul(out, in0, in1)

            def gate_mm(pG, g, nb, passes):
                for mi, (m0, ms) in enumerate(MCH):
                    npass = len(passes)
                    for ki, (rhs, wc, p0, p1) in enumerate(passes):
                        nc.tensor.matmul(
                            pG[0:ms, mi, 0:nb],
                            wt[p0:p1, g, wc, m0:m0 + ms],
                            rhs,
                            start=(ki == 0), stop=(ki == npass - 1))

            def hmul_store(l, c0, nb, ot, tht):
                """h = o*tanh(c) for level-l cols [c0, c0+nb) -> parent f-rhs
                tiles (or hroot), plus the h output DMA."""
                goff = _off(l)
                if l == 0:
                    dst, d0, half = hroot, 0, None
                    stt(dst[:, 0:2, 0:nb], ot[:, 0:2, 0:nb],
                        tht[:, 0:2, 0:nb], ALU.mult)
                    stt(dst[0:44, 2, 0:nb], ot[0:44, 2, 0:nb],
                        tht[0:44, 2, 0:nb], ALU.mult)
                    nc.sync.dma_start(out=HT[0:128, 0:2, goff:goff + nb],
                                      in_=dst[0:128, 0:2, 0:nb])
                    nc.sync.dma_start(out=HT[0:44, 2, goff:goff + nb],
                                      in_=dst[0:44, 2, 0:nb])
                    return
                half = _cols(l - 1)
                # split block at the level's half boundary (if it straddles)
                segs = []
                if c0 < half:
                    seg = min(nb, half - c0)
                    segs.append((mfl[l - 1], c0, 0, seg))
                    if nb > seg:
                        segs.append((mfr[l - 1], 0, seg, nb - seg))
                else:
                    segs.append((mfr[l - 1], c0 - half, 0, nb))
                for dst, d0, s0, sn in segs:
                    stt(dst[:, 1:3, d0:d0 + sn], ot[:, 0:2, s0:s0 + sn],
                        tht[:, 0:2, s0:s0 + sn], ALU.mult)
                    stt(dst[0:44, 0, d0:d0 + sn], ot[0:44, 2, s0:s0 + sn],
                        tht[0:44, 2, s0:s0 + sn], ALU.mult)
                    nc.sync.dma_start(
                        out=HT[0:128, 0:2, goff + c0 + s0:goff + c0 + s0 + sn],
                        in_=dst[0:128, 1:3, d0:d0 + sn])
                    nc.sync.dma_start(
                        out=HT[0:44, 2, goff + c0 + s0:goff + c0 + s0 + sn],
                        in_=dst[0:44, 0, d0:d0 + sn])

            def leaf_block(c0, nb):
                off = _off(LF)
                nc.sync.dma_start(out=lx[:, :, c0:c0 + nb],
                                  in_=X[:, :, off + c0:off + c0 + nb])
                # x-tail + ones rides in unused lanes 44:89 of ct[LF] chunk 2
                nc.sync.dma_start(out=ct[LF][44:109, 2, c0:c0 + nb],
                                  in_=XT[0:65, off + c0:off + c0 + nb])
                passes = [
                    (lx[0:128, 0, c0:c0 + nb], 0, 0, 128),
                    (lx[0:128, 1, c0:c0 + nb], 1, 0, 128),
                    (ct[LF][64:109, 2, c0:c0 + nb], 2, 64, 109),
                ]
                sg = {}
                for g, tg, w_, fn in ((GI, "i", 2, AF.Sigmoid),
                                      (GO, "o", 1, AF.Sigmoid),
                                      (GU, "u", 1, AF.Tanh)):
                    pG = psum.tile([128, 3, NB], F32, tag="g", name="lpg")
                    gate_mm(pG, g, nb, passes)
                    gt = gp.tile([128, 3, w_ * NB], BF16, tag=tg, name="lg")
                    nc.scalar.activation(gt[:, :, 0:nb], pG[:, :, 0:nb], fn)
                    sg[g] = gt
                # c = i*u  (keep lanes 44:89 of chunk2 = x-tail intact)
                cc2 = ct[LF]
                stt(cc2[:, 0:2, c0:c0 + nb], sg[GI][:, 0:2, 0:nb],
                    sg[GU][:, 0:2, 0:nb], ALU.mult)
                stt(cc2[0:44, 2, c0:c0 + nb], sg[GI][0:44, 2, 0:nb],
                    sg[GU][0:44, 2, 0:nb], ALU.mult)
                tht = gp.tile([128, 3, NB], BF16, tag="u", name="lth")
                nc.scalar.activation(tht[:, :, 0:nb],
                                     cc2[:, :, c0:c0 + nb], AF.Tanh)
                hmul_store(LF, c0, nb, sg[GO], tht)

            def inner_block(l, c0, nb):
                half_ch = _cols(l)  # left/right split point in child level
                passes_iou = [
                    (mx[l][0:128, 0, c0:c0 + nb], 0, 0, 128),
                    (mx[l][0:128, 1, c0:c0 + nb], 1, 0, 128),
                    (mio[l][0:109, 0, c0:c0 + nb], 2, 0, 109),
                    (mio[l][0:128, 1, c0:c0 + nb], 3, 0, 128),
                    (mio[l][0:128, 2, c0:c0 + nb], 4, 0, 128),
                ]
                sg = {}
                for g, tg, w_, fn in ((GI, "i", 2, AF.Sigmoid),
                                      (GO, "o", 1, AF.Sigmoid),
                                      (GU, "u", 1, AF.Tanh)):
                    pG = psum.tile([128, 3, NB], F32, tag="g", name="pg")
                    gate_mm(pG, g, nb, passes_iou)
                    gt = gp.tile([128, 3, w_ * NB], BF16, tag=tg, name="gg")
                    nc.scalar.activation(gt[:, :, 0:nb], pG[:, :, 0:nb], fn)
                    sg[g] = gt
                ft = gp.tile([128, 3, 2 * NB], BF16, tag="f", name="fg")
                for s, mf in ((0, mfl[l]), (1, mfr[l])):
                    pF = psum.tile([128, 3, NB], F32, tag="g", name="pf")
                    passes_f = [
                        (mx[l][0:128, 0, c0:c0 + nb], 0, 0, 128),
                        (mx[l][0:128, 1, c0:c0 + nb], 1, 0, 128),
                        (mf[0:109, 0, c0:c0 + nb], 2, 0, 109),
                        (mf[0:128, 1, c0:c0 + nb], 3, 0, 128),
                        (mf[0:128, 2, c0:c0 + nb], 4, 0, 128),
                    ]
                    gate_mm(pF, GF, nb, passes_f)
                    nc.scalar.activation(ft[:, :, s * nb:(s + 1) * nb],
                                         pF[:, :, 0:nb], AF.Sigmoid)
                cc = ct[l][:, :, c0:c0 + nb]
                stt(cc, sg[GI][:, :, 0:nb], sg[GU][:, :, 0:nb], ALU.mult)
                cn = ct[l + 1]
                fct = gp.tile([128, 3, 2 * NB], BF16, tag="i", name="fc")
                stt(fct[:, :, 0:nb], ft[:, :, 0:nb],
                    cn[:, :, c0:c0 + nb], ALU.mult)
                stt(fct[:, :, nb:2 * nb], ft[:, :, nb:2 * nb],
                    cn[:, :, half_ch + c0:half_ch + c0 + nb], ALU.mult)
                stt(cc, cc, fct[:, :, 0:nb], ALU.add)
                stt(cc, cc, fct[:, :, nb:2 * nb], ALU.add)
                tht = gp.tile([128, 3, NB], BF16, tag="u", name="th")
                nc.scalar.activation(tht[:, :, 0:nb], cc, AF.Tanh)
                hmul_store(l, c0, nb, sg[GO], tht)

            def hsum(l, r0, nr):
                """mio[l] h-chunks <- mfl[l] + mfr[l] over cols [r0, r0+nr)."""
                stt(mio[l][:, 1:3, r0:r0 + nr], mfl[l][:, 1:3, r0:r0 + nr],
                    mfr[l][:, 1:3, r0:r0 + nr], ALU.add)
                stt(mio[l][0:44, 0, r0:r0 + nr], mfl[l][0:44, 0, r0:r0 + nr],
                    mfr[l][0:44, 0, r0:r0 + nr], ALU.add)

            def emit_level_inputs(l):
                off = _off(l)
                nc.sync.dma_start(out=mx[l][:, :, :],
                                  in_=X[:, :, off:off + _cols(l)])
                for m in (mio[l], mfl[l], mfr[l]):
                    nc.sync.dma_start(out=m[44:109, 0, 0:_cols(l)],
                                      in_=XT[0:65, off:off + _cols(l)])

            # ---------------- leaves ----------------
            nblk = _cols(LF) // NB
            for b in range(nblk // 2):
                leaf_block(b * NB, NB)
                leaf_block((b + nblk // 2) * NB, NB)
                if b == 0:
                    emit_level_inputs(LF - 1)
                hsum(LF - 1, b * NB, NB)

            # ---------------- internal levels ----------------
            for l in range(LF - 1, -1, -1):
                if l > 0:
                    emit_level_inputs(l - 1)
                cols = _cols(l)
                if cols >= 2 * NB:
                    nblk = cols // NB
                    for b in range(nblk // 2):
                        inner_block(l, b * NB, NB)
                        inner_block(l, (b + nblk // 2) * NB, NB)
                        if l > 0:
                            hsum(l - 1, b * NB, NB)
                else:
                    inner_block(l, 0, cols)
                    if l > 0:
                        hsum(l - 1, 0, cols // 2)
    nc.compile()
    return nc


def _bitrev(x, bits):
    r = np.zeros_like(x)
    for i in range(bits):
        r = (r << 1) | ((x >> i) & 1)
    return r


def _colmaps():
    if "maps" in _PREP_CACHE:
        return _PREP_CACHE["maps"]
    node_of_col = np.empty(NCOLS, np.int64)
    tree_of_col = np.empty(NCOLS, np.int64)
    for l in range(D):
        n = 1 << l
        off = _off(l)
        r = np.arange(n)
        j = _bitrev(r, l)
        nodes = (n - 1) + j                      # per-tree node index
        cols = off + r[:, None] * BL + np.arange(BL)[None, :]
        node_of_col[cols.ravel()] = np.repeat(nodes, BL)
        tree_of_col[cols.ravel()] = np.tile(np.arange(BL), n)
    flat = tree_of_col * N + node_of_col
    _PREP_CACHE["maps"] = (node_of_col, tree_of_col, flat)
    return _PREP_CACHE["maps"]


def kernel(embs, Wix, bix, Wih, bih, Wfx, bfx, Wfh, bfh,
           Wox, box, Woh, boh, Wux, bux, Wuh, buh):
    embs = np.ascontiguousarray(np.asarray(embs, dtype=np.float32))
    if not _NC_CACHE:
        _NC_CACHE.append(_build())
    nc = _NC_CACHE[0]
    node_of_col, tree_of_col, flat = _colmaps()

    w = np.zeros((128, 4, 5, DIM), np.float32)
    gates = [(Wix, bix, Wih, bih), (Wox, box, Woh, boh),
             (Wux, bux, Wuh, buh), (Wfx, bfx, Wfh, bfh)]
    for gi, (Wx, bx, Wh, bh) in enumerate(gates):
        Wx = np.asarray(Wx, np.float32)
        Wh = np.asarray(Wh, np.float32)
        w[:, gi, 0] = Wx[0:128]
        w[:, gi, 1] = Wx[128:256]
        w[0:44, gi, 2] = Wh[256:300]
        w[64:108, gi, 2] = Wx[256:300]
        w[108, gi, 2] = np.asarray(bx, np.float32) + np.asarray(bh, np.float32)
        w[:, gi, 3] = Wh[0:128]
        w[:, gi, 4] = Wh[128:256]
    wp = w.astype(ml_dtypes.bfloat16)

    ones_row = np.ones((1, NCOLS), np.float32)
    zero_rows = np.zeros((20, NCOLS), np.float32)
    in_maps = []
    for c in range(CORES):
        exf = np.ascontiguousarray(
            embs[c * BL:(c + 1) * BL].transpose(2, 0, 1)).reshape(DIM, BL * N)
        colsv = exf[:, flat]                     # [300, NCOLS]
        xa = np.ascontiguousarray(
            colsv[0:256].reshape(2, 128, NCOLS).transpose(1, 0, 2)
        ).astype(ml_dtypes.bfloat16)
        xta = np.ascontiguousarray(
            np.concatenate([zero_rows, colsv[256:300], ones_row], 0)
        ).astype(ml_dtypes.bfloat16)
        in_maps.append({"x": xa, "xt": xta, "w": wp})

    res = run_bass_kernel_spmd(nc, in_maps, list(range(CORES)))

    out = np.empty((B, N, DIM), np.float32)
    hT = np.empty((DIM, NCOLS), np.float32)
    for c in range(CORES):
        ht = np.asarray(res.results[c]["ht"]).astype(np.float32)
        hT[0:128] = ht[:, 0]
        hT[128:256] = ht[:, 1]
        hT[256:300] = ht[0:44, 2]
        out[c * BL + tree_of_col, node_of_col] = hT.T
    return out


# revision 17
# speedup vs baseline: 2.1342x; 1.2240x over previous
"""Child-Sum TreeLSTM over complete binary trees — Trainium2 Bass kernel.

Sharding: data-parallel over the batch-of-trees axis B=32 across 8 NeuronCores
(4 trees/core); the 8 gate weight matrices are replicated.

Layout: per level, columns are permuted by per-level bit-reversal (col =
bitrev_l(j)*4 + tree).  Under this order the children of parent col p at
level l sit at cols p (left) and cols_l + p (right) of level l+1 — children
are split into two contiguous halves, so child-sum, per-child forget gates
and f*c reductions are all plain packed slices (no strided pairs, no
broadcast rhs).  The host prepares feature-major bf16 copies of embs in this
order and un-permutes h on the way out, so the kernel does no transposes.

Per-core dataflow, all matmuls bf16 (1 cycle/col):
  rhs tiles per level: X (x rows 0..255), and three M tiles (iou / f-left /
  f-right) holding chunk0 = [h rows 256..299 | x rows 256..299 | ones] and
  chunks 1,2 = h rows 0..127 / 128..255.  The ones lane carries the combined
  bias (bx+bh) as a weight row, so gate preacts finish in 5 matmul passes
  (3 for leaves) per (gate, m-chunk), PSUM-accumulated.
  PSUM evacuation with sigmoid/tanh on ACT; all elementwise work uses
  scalar_tensor_tensor (InstTensorScalarPtr, 4x DVE perf mode on packed
  bf16 SBUF).  h is written straight into the parent level's f-rhs tiles;
  h_sum is one packed add.  Everything stays SBUF-resident (no spills).
"""

import numpy as np
import ml_dtypes

import concourse.bass as bass
import concourse.mybir as mybir
import concourse.tile as tile
from concourse import bacc
from concourse.bass_utils import run_bass_kernel_spmd

F32 = mybir.dt.float32
BF16 = mybir.dt.bfloat16
AF = mybir.ActivationFunctionType
ALU = mybir.AluOpType

B, D, DIM = 32, 11, 300
N = 2**D - 1            # 2047
CORES = 8
BL = B // CORES         # 4 trees per core
NCOLS = BL * N          # 8188
NB = 512
LF = D - 1              # leaf level = 10
MCH = ((0, 128), (128, 128), (256, 44))   # m-chunks of 300

_NC_CACHE = []
_PREP_CACHE = {}


def _cols(l):
    return BL * (1 << l)


def _off(l):
    return BL * ((1 << l) - 1)


def _build():
    nc = bacc.Bacc("TRN2", target_bir_lowering=False, debug=False,
                   num_devices=CORES)
    X = nc.dram_tensor("x", [128, 2, NCOLS], BF16, kind="ExternalInput")
    XT = nc.dram_tensor("xt", [84, NCOLS], BF16, kind="ExternalInput")
    W = nc.dram_tensor("w", [128, 4, 5, DIM], BF16, kind="ExternalInput")
    W2 = nc.dram_tensor("w2", [128, 5, 128], BF16, kind="ExternalInput")
    HT = nc.dram_tensor("ht", [128, 3, NCOLS], BF16, kind="ExternalOutput")

    GI, GO, GU, GF = 0, 1, 2, 3

    with tile.TileContext(nc) as tc:
        import contextlib
        with contextlib.ExitStack() as ctx:
            sb = ctx.enter_context(tc.tile_pool(name="sb", bufs=1))
            gp = ctx.enter_context(tc.tile_pool(name="gp", bufs=2))
            psum = ctx.enter_context(
                tc.tile_pool(name="psum", bufs=2, space="PSUM"))

            wt = sb.tile([128, 4, 5, DIM], BF16, name="wt")
            w2t = sb.tile([128, 5, 128], BF16, name="w2t")
            for g in (0, 2, 1):   # i, u, o — all used by the first leaf block
                nc.sync.dma_start(out=wt[:, g], in_=W[:, g])

            # per-level tiles
            mx = {l: sb.tile([128, 2, _cols(l)], BF16, name=f"mx{l}",
                             tag=f"mx{l}") for l in range(0, LF)}
            mio = {l: sb.tile([128, 3, _cols(l)], BF16, name=f"mio{l}",
                              tag=f"mio{l}") for l in range(0, LF)}
            mfl = {l: sb.tile([128, 3, _cols(l)], BF16, name=f"mfl{l}",
                              tag=f"mfl{l}") for l in range(0, LF)}
            mfr = {l: sb.tile([128, 3, _cols(l)], BF16, name=f"mfr{l}",
                              tag=f"mfr{l}") for l in range(0, LF)}
            lx = sb.tile([128, 2, _cols(LF)], BF16, name="lx")
            ct = {l: sb.tile([128, 3, _cols(l)], BF16, name=f"ct{l}",
                             tag=f"ct{l}") for l in range(0, LF + 1)}
            hroot = sb.tile([128, 3, BL], BF16, name="hroot")

            for tg, wd in (("i", 2), ("u", 1)):
                for _ in range(2):
                    t0 = gp.tile([128, 3, wd * NB], BF16, tag=tg, name="zi")
                    nc.gpsimd.memset(t0[:, 2, :], 0.0)

            def stt(out, in0, in1, op1):
                # tensor_tensor gets the 2x_1p DVE perf mode on packed bf16
                # (scalar_tensor_tensor supports no perf modes)
                if op1 == ALU.add:
                    nc.vector.tensor_add(out, in0, in1)
                else:
                    nc.vector.tensor_mul(out, in0, in1)

            def gate_mm(pG, g, nb, xpasses, hpasses=(), mch=MCH):
                # x-side passes for every m-chunk first (they depend only on
                # DMA'd inputs), h-side passes after — so the tensor engine
                # can run ahead into the next level while the child level's
                # evac/c/h chain drains.
                for mi, (m0, ms) in enumerate(mch):
                    for ki, (rhs, wc, p0, p1) in enumerate(xpasses):
                        nc.tensor.matmul(
                            pG[0:ms, mi, 0:nb],
                            wt[p0:p1, g, wc, m0:m0 + ms],
                            rhs,
                            start=(ki == 0),
                            stop=(not hpasses and ki == len(xpasses) - 1))
                for mi, (m0, ms) in enumerate(mch):
                    for ki, (rhs, wc, p0, p1) in enumerate(hpasses):
                        nc.tensor.matmul(
                            pG[0:ms, mi, 0:nb],
                            wt[p0:p1, g, wc, m0:m0 + ms],
                            rhs,
                            start=False, stop=(ki == len(hpasses) - 1))

            def tail_mm(pT, nb, xpasses, hpasses):
                # packed m-tail pass: out partitions 0:44 = i-tail rows
                # 256:300, 64:108 = u-tail rows (one matmul streams both)
                for ki, (rhs, wc, p0, p1) in enumerate(xpasses):
                    nc.tensor.matmul(pT[0:108, 0:nb], w2t[p0:p1, wc, 0:108],
                                     rhs, start=(ki == 0), stop=False)
                for ki, (rhs, wc, p0, p1) in enumerate(hpasses):
                    nc.tensor.matmul(pT[0:108, 0:nb], w2t[p0:p1, wc, 0:108],
                                     rhs, start=False,
                                     stop=(ki == len(hpasses) - 1))

            def hmul_store(l, c0, nb, ot, tht):
                """h = o*tanh(c) for level-l cols [c0, c0+nb) -> parent f-rhs
                tiles (or hroot), plus the h output DMA."""
                goff = _off(l)
                odma = nc.gpsimd.dma_start if l >= 8 else nc.sync.dma_start
                if l == 0:
                    dst, d0, half = hroot, 0, None
                    stt(dst[:, 0:2, 0:nb], ot[:, 0:2, 0:nb],
                        tht[:, 0:2, 0:nb], ALU.mult)
                    stt(dst[0:44, 2, 0:nb], ot[0:44, 2, 0:nb],
                        tht[0:44, 2, 0:nb], ALU.mult)
                    odma(out=HT[0:128, 0:2, goff:goff + nb],
                         in_=dst[0:128, 0:2, 0:nb])
                    odma(out=HT[0:44, 2, goff:goff + nb],
                         in_=dst[0:44, 2, 0:nb])
                    return
                half = _cols(l - 1)
                # split block at the level's half boundary (if it straddles)
                segs = []
                if c0 < half:
                    seg = min(nb, half - c0)
                    segs.append((mfl[l - 1], c0, 0, seg))
                    if nb > seg:
                        segs.append((mfr[l - 1], 0, seg, nb - seg))
                else:
                    segs.append((mfr[l - 1], c0 - half, 0, nb))
                for dst, d0, s0, sn in segs:
                    stt(dst[:, 1:3, d0:d0 + sn], ot[:, 0:2, s0:s0 + sn],
                        tht[:, 0:2, s0:s0 + sn], ALU.mult)
                    stt(dst[0:44, 0, d0:d0 + sn], ot[0:44, 2, s0:s0 + sn],
                        tht[0:44, 2, s0:s0 + sn], ALU.mult)
                    odma(
                        out=HT[0:128, 0:2, goff + c0 + s0:goff + c0 + s0 + sn],
                        in_=dst[0:128, 1:3, d0:d0 + sn])
                    odma(
                        out=HT[0:44, 2, goff + c0 + s0:goff + c0 + s0 + sn],
                        in_=dst[0:44, 0, d0:d0 + sn])

            def leaf_block(c0, nb, eng=None):
                off = _off(LF)
                eng = eng or nc.sync
                eng.dma_start(out=lx[:, :, c0:c0 + nb],
                              in_=X[:, :, off + c0:off + c0 + nb])
                # x-tail + ones rides in unused lanes 44:109 of ct[LF] chunk 2
                eng.dma_start(out=ct[LF][44:109, 2, c0:c0 + nb],
                              in_=XT[0:65, off + c0:off + c0 + nb])
                passes = [
                    (lx[0:128, 0, c0:c0 + nb], 0, 0, 128),
                    (lx[0:128, 1, c0:c0 + nb], 1, 0, 128),
                    (ct[LF][64:109, 2, c0:c0 + nb], 2, 64, 109),
                ]
                sg = {}
                for g, tg, w_, fn in ((GI, "i", 2, AF.Sigmoid),
                                      (GU, "u", 1, AF.Tanh),
                                      (GO, "o", 1, AF.Sigmoid)):
                    pG = psum.tile([128, 3, NB], F32, tag="g", name="lpg")
                    gate_mm(pG, g, nb, passes)
                    gt = gp.tile([128, 3, w_ * NB], BF16, tag=tg, name="lg")
                    nc.scalar.activation(gt[:, :, 0:nb], pG[:, :, 0:nb], fn)
                    sg[g] = gt
                    if g == GU:
                        # c = i*u (keep lanes 44:109 of chunk2 = x-tail)
                        cc2 = ct[LF]
                        stt(cc2[:, 0:2, c0:c0 + nb], sg[GI][:, 0:2, 0:nb],
                            sg[GU][:, 0:2, 0:nb], ALU.mult)
                        stt(cc2[0:44, 2, c0:c0 + nb], sg[GI][0:44, 2, 0:nb],
                            sg[GU][0:44, 2, 0:nb], ALU.mult)
                        tht = gp.tile([128, 3, NB], BF16, tag="u", name="lth")
                        nc.scalar.activation(tht[:, :, 0:nb],
                                             cc2[:, :, c0:c0 + nb], AF.Tanh)
                hmul_store(LF, c0, nb, sg[GO], tht)

            def inner_block(l, c0, nb):
                half_ch = _cols(l)  # left/right split point in child level
                xp = [
                    (mx[l][0:128, 0, c0:c0 + nb], 0, 0, 128),
                    (mx[l][0:128, 1, c0:c0 + nb], 1, 0, 128),
                ]
                # forget gates first: their evac -> f*c chain is the long
                # pole, so start it as early as possible
                ft = gp.tile([128, 3, 2 * NB], BF16, tag="f", name="fg")
                for s, mf in ((0, mfl[l]), (1, mfr[l])):
                    pF = psum.tile([128, 3, NB], F32, tag="g", name="pf")
                    hp = [
                        (mf[0:109, 0, c0:c0 + nb], 2, 0, 109),
                        (mf[0:128, 1, c0:c0 + nb], 3, 0, 128),
                        (mf[0:128, 2, c0:c0 + nb], 4, 0, 128),
                    ]
                    gate_mm(pF, GF, nb, xp, hp)
                    nc.scalar.activation(ft[:, :, s * nb:(s + 1) * nb],
                                         pF[:, :, 0:nb], AF.Sigmoid)
                cn = ct[l + 1]
                fct = gp.tile([128, 3, 2 * NB], BF16, tag="i", name="fc")
                stt(fct[:, :, 0:nb], ft[:, :, 0:nb],
                    cn[:, :, c0:c0 + nb], ALU.mult)
                stt(fct[:, :, nb:2 * nb], ft[:, :, nb:2 * nb],
                    cn[:, :, half_ch + c0:half_ch + c0 + nb], ALU.mult)
                hp_io = [
                    (mio[l][0:109, 0, c0:c0 + nb], 2, 0, 109),
                    (mio[l][0:128, 1, c0:c0 + nb], 3, 0, 128),
                    (mio[l][0:128, 2, c0:c0 + nb], 4, 0, 128),
                ]
                sg = {}
                cc = ct[l][:, :, c0:c0 + nb]
                MCH2 = MCH[0:2]
                for g, tg, w_, fn in ((GI, "i", 2, AF.Sigmoid),
                                      (GU, "u", 1, AF.Tanh)):
                    pG = psum.tile([128, 3, NB], F32, tag="g", name="pg")
                    gate_mm(pG, g, nb, xp, hp_io, mch=MCH2)
                    gt = gp.tile([128, 3, w_ * NB], BF16, tag=tg, name="gg")
                    nc.scalar.activation(gt[:, 0:2, 0:nb], pG[:, 0:2, 0:nb],
                                         fn)
                    sg[g] = gt
                # packed i+u m-tail (rows 256:300 of both gates, one stream)
                pT = psum.tile([128, NB], F32, tag="t", bufs=1, name="pt")
                tail_mm(pT, nb, xp, hp_io)
                nc.scalar.activation(sg[GI][0:44, 2, 0:nb], pT[0:44, 0:nb],
                                     AF.Sigmoid)
                nc.scalar.activation(sg[GU][0:44, 2, 0:nb], pT[64:108, 0:nb],
                                     AF.Tanh)
                stt(cc, sg[GI][:, :, 0:nb], sg[GU][:, :, 0:nb], ALU.mult)
                stt(cc, cc, fct[:, :, 0:nb], ALU.add)
                stt(cc, cc, fct[:, :, nb:2 * nb], ALU.add)
                tht = gp.tile([128, 3, NB], BF16, tag="u", name="th")
                nc.scalar.activation(tht[:, :, 0:nb], cc, AF.Tanh)
                pG = psum.tile([128, 3, NB], F32, tag="g", name="pgo")
                gate_mm(pG, GO, nb, xp, hp_io)
                ot = gp.tile([128, 3, NB], BF16, tag="o", name="og")
                nc.scalar.activation(ot[:, :, 0:nb], pG[:, :, 0:nb],
                                     AF.Sigmoid)
                sg[GO] = ot
                hmul_store(l, c0, nb, sg[GO], tht)

            def hsum(l, r0, nr):
                """mio[l] h-chunks <- mfl[l] + mfr[l] over cols [r0, r0+nr)."""
                stt(mio[l][:, 1:3, r0:r0 + nr], mfl[l][:, 1:3, r0:r0 + nr],
                    mfr[l][:, 1:3, r0:r0 + nr], ALU.add)
                stt(mio[l][0:44, 0, r0:r0 + nr], mfl[l][0:44, 0, r0:r0 + nr],
                    mfr[l][0:44, 0, r0:r0 + nr], ALU.add)

            def emit_level_inputs(l):
                off = _off(l)
                nc.sync.dma_start(out=mx[l][:, :, :],
                                  in_=X[:, :, off:off + _cols(l)])
                for m in (mio[l], mfl[l], mfr[l]):
                    nc.sync.dma_start(out=m[44:128, 0, 0:_cols(l)],
                                      in_=XT[0:84, off:off + _cols(l)])

            # ---------------- leaves ----------------
            nblk = _cols(LF) // NB
            for b in range(nblk // 2):
                leaf_block(b * NB, NB, eng=nc.scalar if b == 0 else None)
                if b == 0:
                    nc.sync.dma_start(out=wt[:, 3], in_=W[:, 3])
                    nc.sync.dma_start(out=w2t[:, :, :], in_=W2[:, :, :])
                leaf_block((b + nblk // 2) * NB, NB)
                if b == 0:
                    emit_level_inputs(LF - 1)
                hsum(LF - 1, b * NB, NB)

            # ---------------- internal levels ----------------
            for l in range(LF - 1, -1, -1):
                if l > 0:
                    emit_level_inputs(l - 1)
                cols = _cols(l)
                if cols >= 2 * NB:
                    nblk = cols // NB
                    for b in range(nblk // 2):
                        inner_block(l, b * NB, NB)
                        inner_block(l, (b + nblk // 2) * NB, NB)
                        if l > 0:
                            hsum(l - 1, b * NB, NB)
                else:
                    inner_block(l, 0, cols)
                    if l > 0:
                        hsum(l - 1, 0, cols // 2)
    nc.compile()
    return nc


def _bitrev(x, bits):
    r = np.zeros_like(x)
    for i in range(bits):
        r = (r << 1) | ((x >> i) & 1)
    return r


def _colmaps():
    if "maps" in _PREP_CACHE:
        return _PREP_CACHE["maps"]
    node_of_col = np.empty(NCOLS, np.int64)
    tree_of_col = np.empty(NCOLS, np.int64)
    for l in range(D):
        n = 1 << l
        off = _off(l)
        r = np.arange(n)
        j = _bitrev(r, l)
        nodes = (n - 1) + j                      # per-tree node index
        cols = off + r[:, None] * BL + np.arange(BL)[None, :]
        node_of_col[cols.ravel()] = np.repeat(nodes, BL)
        tree_of_col[cols.ravel()] = np.tile(np.arange(BL), n)
    flat = tree_of_col * N + node_of_col
    _PREP_CACHE["maps"] = (node_of_col, tree_of_col, flat)
    return _PREP_CACHE["maps"]


def kernel(embs, Wix, bix, Wih, bih, Wfx, bfx, Wfh, bfh,
           Wox, box, Woh, boh, Wux, bux, Wuh, buh):
    embs = np.ascontiguousarray(np.asarray(embs, dtype=np.float32))
    if not _NC_CACHE:
        _NC_CACHE.append(_build())
    nc = _NC_CACHE[0]
    node_of_col, tree_of_col, flat = _colmaps()

    w = np.zeros((128, 4, 5, DIM), np.float32)
    gates = [(Wix, bix, Wih, bih), (Wox, box, Woh, boh),
             (Wux, bux, Wuh, buh), (Wfx, bfx, Wfh, bfh)]
    for gi, (Wx, bx, Wh, bh) in enumerate(gates):
        Wx = np.asarray(Wx, np.float32)
        Wh = np.asarray(Wh, np.float32)
        w[:, gi, 0] = Wx[0:128]
        w[:, gi, 1] = Wx[128:256]
        w[0:44, gi, 2] = Wh[256:300]
        w[64:108, gi, 2] = Wx[256:300]
        w[108, gi, 2] = np.asarray(bx, np.float32) + np.asarray(bh, np.float32)
        w[:, gi, 3] = Wh[0:128]
        w[:, gi, 4] = Wh[128:256]
    wp = w.astype(ml_dtypes.bfloat16)

    w2 = np.zeros((128, 5, 128), np.float32)
    for col0, (Wx, bx, Wh, bh) in ((0, gates[0]), (64, gates[2])):  # i, u
        Wx = np.asarray(Wx, np.float32)
        Wh = np.asarray(Wh, np.float32)
        w2[:, 0, col0:col0 + 44] = Wx[0:128, 256:300]
        w2[:, 1, col0:col0 + 44] = Wx[128:256, 256:300]
        w2[0:44, 2, col0:col0 + 44] = Wh[256:300, 256:300]
        w2[64:108, 2, col0:col0 + 44] = Wx[256:300, 256:300]
        w2[108, 2, col0:col0 + 44] = (np.asarray(bx, np.float32)
                                      + np.asarray(bh, np.float32))[256:300]
        w2[:, 3, col0:col0 + 44] = Wh[0:128, 256:300]
        w2[:, 4, col0:col0 + 44] = Wh[128:256, 256:300]
    w2p = w2.astype(ml_dtypes.bfloat16)

    ones_row = np.ones((1, NCOLS), np.float32)
    zero_rows = np.zeros((20, NCOLS), np.float32)
    zero_tail = np.zeros((19, NCOLS), np.float32)
    in_maps = []
    for c in range(CORES):
        exf = np.ascontiguousarray(
            embs[c * BL:(c + 1) * BL].transpose(2, 0, 1)).reshape(DIM, BL * N)
        colsv = exf[:, flat]                     # [300, NCOLS]
        xa = np.ascontiguousarray(
            colsv[0:256].reshape(2, 128, NCOLS).transpose(1, 0, 2)
        ).astype(ml_dtypes.bfloat16)
        xta = np.ascontiguousarray(
            np.concatenate([zero_rows, colsv[256:300], ones_row, zero_tail],
                           0)
        ).astype(ml_dtypes.bfloat16)
        in_maps.append({"x": xa, "xt": xta, "w": wp, "w2": w2p})

    res = run_bass_kernel_spmd(nc, in_maps, list(range(CORES)))

    out = np.empty((B, N, DIM), np.float32)
    hT = np.empty((DIM, NCOLS), np.float32)
    for c in range(CORES):
        ht = np.asarray(res.results[c]["ht"]).astype(np.float32)
        hT[0:128] = ht[:, 0]
        hT[128:256] = ht[:, 1]
        hT[256:300] = ht[0:44, 2]
        out[c * BL + tree_of_col, node_of_col] = hT.T
    return out
